# revision 7
# baseline (speedup 1.0000x reference)
"""Trainium2 Bass kernel for the 4-layer LSTM (T=128, B=64, H=1024).

Strategy: 4-stage layer pipeline x 2-way batch data-parallel = 8 cores
(rank r: stage j = r % 4, batch half r // 4), with the per-step matmuls
in "streaming" form: the STATIONARY operand is the (tiny) transposed
activation tile xT/hT [128, 32] and the MOVING operand is the weight
matrix, 4-way column-tiled across the PE array by gate. This replaces
the baseline's 256 LDWEIGHTS-bound [128x128] weight loads per step
(~34ns each, N=32) with 64 N=512 streaming matmuls running 4-concurrent
(~3.4us/step for h@U instead of ~8.7us).

z lands in PSUM gate-major: partition 32*gate + b, free dim = 1024
h-dims. Gates run on Scalar/Vector/GpSimd with partition-shifted
reads (z_i at 0:31, z_f at 32:63, ...), cell state c lives in PSUM at
partitions 32:64 so f*c_prev reads PSUM (mixed-space operands may
differ in base partition; SBUF+SBUF pairs must match - all gate
intermediates are written to base 0). h and c are transposed back to
[128, 32] k-tile blocks via matmul-against-identity (8+8 tiny MMs/step)
to form the next step's stationary hT and the transport chunk cT.

Transport: 4-rank intra-chip AllGather ([[0-3],[4-7]]) once per 3-tick
round (6 steps) - collective-chain ops are latency-bound (~20-35us
unloaded, ~57us spacing when saturated), so one op per ~50us of compute
keeps the chain unloaded. 3-deep output rotation + 3-deep c_out
double-buffering give multi-tick safe read/write windows. Stage lag
OFF=7 ticks.

Output = cell state of layer 3 (ranks 3/7) read directly from PSUM in
[batch, hidden] layout - no host transpose.
"""

import sys

for p in ("/opt/trn_rl_repo",):
    if p not in sys.path:
        sys.path.insert(0, p)

import numpy as np
import ml_dtypes

T, B, H, L = 128, 64, 1024, 4
KT = H // 128            # 8 k-tiles over the contraction dim
B_LOC = B // 2           # batch per core
G = 2                    # steps per tick (transport granularity)
NCH = T // G             # 64 chunks per layer
R = 3                    # ticks per gather round
OFF = 7                  # tick offset between consecutive stages
C0 = 2                   # first active tick of stage 0
NTICKS = NCH + OFF * (L - 1) + C0
N_CORES = 8
CW = G * KT * B_LOC      # cols per chunk in cT layout (= 512)
SW = KT * B_LOC          # cols per step (= 256)

_CACHE = {}


def _build(nticks=NTICKS):
    import concourse.bacc as bacc
    import concourse.bass as bass
    import concourse.mybir as mybir
    import concourse.tile as tile

    bf16, f32, i32 = mybir.dt.bfloat16, mybir.dt.float32, mybir.dt.int32
    AF = mybir.ActivationFunctionType
    Alu = mybir.AluOpType

    nc = bacc.Bacc("TRN2", target_bir_lowering=False, debug=False,
                   num_devices=N_CORES)

    # weights, k-tile k at cols k*4H (moving operand layout)
    w_in = nc.dram_tensor("w_loc", [128, KT * 4 * H], bf16,
                          kind="ExternalInput")
    u_in = nc.dram_tensor("u_loc", [128, KT * 4 * H], bf16,
                          kind="ExternalInput")
    # layer-0 input, xT chunks: [128, chunk, step, k, b]
    src_static = nc.dram_tensor("src_static", [128, NCH * CW], bf16,
                                kind="ExternalInput")
    eye_in = nc.dram_tensor("eye", [32, 32], bf16, kind="ExternalInput")
    rparam = nc.dram_tensor("rparam", [1, 1], i32, kind="ExternalInput")
    out_ext = nc.dram_tensor("out", [32, H], f32, kind="ExternalOutput")

    # transport round buffers (a round = R ticks of cT chunks)
    c_out = [nc.dram_tensor(f"c_out{i}", [128, R * CW], bf16)
             for i in range(3)]
    gbufs = [nc.dram_tensor(f"gath{i}", [4, 128, R * CW], bf16)
             for i in range(3)]

    with tile.TileContext(nc) as tc:
        with (
            tc.tile_pool(name="wp", bufs=1) as wp,
            tc.tile_pool(name="sp", bufs=1) as sp,
            tc.tile_pool(name="srcp", bufs=3) as srcp,
            tc.tile_pool(name="ewp", bufs=2) as ewp,
            tc.tile_pool(name="zp", bufs=1, space="PSUM") as zp,
            tc.tile_pool(name="tp", bufs=2, space="PSUM") as tp_,
        ):
            # ---- preamble -------------------------------------------------
            w_sb = wp.tile([128, KT * 4 * H], bf16)
            u_sb = wp.tile([128, KT * 4 * H], bf16)
            nc.sync.dma_start(w_sb[:], w_in[:])
            nc.sync.dma_start(u_sb[:], u_in[:])

            eye_sb = sp.tile([32, 32], bf16)
            nc.sync.dma_start(eye_sb[:], eye_in[:])

            rp_sb = sp.tile([1, 1], i32)
            nc.sync.dma_start(rp_sb[:], rparam[:])
            rv = nc.values_load(rp_sb[:1, 0:1].to_broadcast((1, 1)))

            zsb = sp.tile([128, R * CW], bf16)
            nc.gpsimd.memset(zsb[:], 0.0)
            for cb in c_out:
                nc.sync.dma_start(cb[:, :], zsb[:])
            for gb in gbufs:
                for s in range(4):
                    nc.sync.dma_start(gb[s][:, :], zsb[:])

            # persistent state
            hT = [sp.tile([128, SW], bf16, name=f"hT{i}") for i in range(2)]
            for i in range(2):
                nc.gpsimd.memset(hT[i][:], 0.0)
            z_ps = [zp.tile([128, H], f32, name=f"z{i}") for i in range(2)]
            c_ps = zp.tile([128, H], f32, name="c_ps")
            nc.vector.memset(c_ps[32:64, :], 0.0)

            # src chunk for consuming tick tc_ (issued 2 ticks early)
            def issue_src(tc_):
                src_sb = srcp.tile([128, CW], bf16, tag="src",
                                   name=f"src_{tc_}")
                kchunk = min(max(tc_ - C0, 0), NCH - 1)
                tp = max(tc_ - OFF, 0)
                m_g = tp // R
                sub = tp % R
                sel = m_g % 3
                gt = gbufs[sel]
                with tc.If(rv == 0) as cmp:
                    nc.sync.dma_start(
                        src_sb[:],
                        src_static[:, kchunk * CW:(kchunk + 1) * CW])
                with cmp.Else():
                    for j in (1, 2, 3):
                        with tc.If(rv == j):
                            nc.sync.dma_start(
                                src_sb[:],
                                gt[j - 1][:, sub * CW:(sub + 1) * CW])
                return src_sb

            # x@W streaming matmuls for step slot s of src tile -> z buf
            def issue_xw(zt, src_t, s):
                for nh in range(2):
                    for k in range(KT):
                        for g in range(4):
                            nc.tensor.matmul(
                                zt[32 * g:32 * (g + 1),
                                   nh * 512:(nh + 1) * 512],
                                src_t[:, (s % G) * SW + 32 * k:
                                      (s % G) * SW + 32 * (k + 1)],
                                w_sb[:, k * 4 * H + g * H + nh * 512:
                                     k * 4 * H + g * H + (nh + 1) * 512],
                                start=(k == 0), stop=False,
                                skip_group_check=True,
                                tile_position=(0, 32 * g),
                            )

            def issue_hu(zt, hT_t):
                for nh in range(2):
                    for k in range(KT):
                        for g in range(4):
                            nc.tensor.matmul(
                                zt[32 * g:32 * (g + 1),
                                   nh * 512:(nh + 1) * 512],
                                hT_t[:, 32 * k:32 * (k + 1)],
                                u_sb[:, k * 4 * H + g * H + nh * 512:
                                     k * 4 * H + g * H + (nh + 1) * 512],
                                start=False,
                                stop=(k == KT - 1),
                                skip_group_check=True,
                                tile_position=(0, 32 * g),
                            )

            gstep = 0
            srcs = {0: issue_src(0), 1: issue_src(1)}
            issue_xw(z_ps[0], srcs[0], 0)

            # ---- tick loop ------------------------------------------------
            for tau in range(nticks):
                if tau % R == 0 and tau > 0:
                    m = tau // R
                    nc.gpsimd.collective_compute(
                        "AllGather", Alu.bypass,
                        replica_groups=[[0, 1, 2, 3], [4, 5, 6, 7]],
                        ins=[c_out[(m - 1) % 3].ap().opt()],
                        outs=[gbufs[(m - 1) % 3].ap().opt()],
                    )

                # state reset at each stage's first active tick
                if tau >= C0 and (tau - C0) % OFF == 0 and (tau - C0) // OFF < L:
                    j = (tau - C0) // OFF
                    with tc.If(rv == j):
                        nc.gpsimd.memset(hT[0][:], 0.0)
                        nc.gpsimd.memset(hT[1][:], 0.0)
                        nc.vector.memset(c_ps[32:64, :], 0.0)

                if tau + 2 < nticks:
                    srcs[tau + 2] = issue_src(tau + 2)
                last = tau == nticks - 1

                cbf = ewp.tile([128, CW], bf16, tag="cbf", name=f"cbf_{tau}")
                for s in range(G):
                    zt = z_ps[gstep % 2]
                    # h @ U accumulated on top of x@W prefill (nh=0 first so
                    # the gate chain on z[:, :512] starts mid-burst)
                    issue_hu(zt, hT[(gstep + 1) % 2])

                    # gates, pipelined by 512-col n-half across engines.
                    # z partitions: i 0:32, f 32:64, g 64:96, o 96:128.
                    sif = ewp.tile([64, H], f32, tag="sif",
                                   name=f"sif_{tau}_{s}")
                    tg = ewp.tile([32, H], f32, tag="tg", name=f"tg_{tau}_{s}")
                    so = ewp.tile([32, H], f32, tag="so", name=f"so_{tau}_{s}")
                    fc = ewp.tile([32, H], f32, tag="fc", name=f"fc_{tau}_{s}")
                    ig = ewp.tile([32, H], f32, tag="ig", name=f"ig_{tau}_{s}")
                    th = ewp.tile([32, H], f32, tag="th", name=f"th_{tau}_{s}")
                    h_bf = ewp.tile([32, H], bf16, tag="hbf",
                                    name=f"hbf_{tau}_{s}")
                    c_bf = ewp.tile([32, H], bf16, tag="cbfc",
                                    name=f"cbfc_{tau}_{s}")

                    def nhs(t, nh):
                        return t[:, nh * 512:(nh + 1) * 512]

                    # scalar (ACT) program order: all z-acts for both halves
                    # first, then the c-dependent tanh per half.
                    for nh in range(2):
                        nc.scalar.activation(nhs(sif, nh), zt[0:64, nh * 512:
                                                              (nh + 1) * 512],
                                             AF.Sigmoid)
                        nc.scalar.activation(nhs(tg, nh), zt[64:96, nh * 512:
                                                             (nh + 1) * 512],
                                             AF.Tanh)
                        nc.scalar.activation(nhs(so, nh), zt[96:128, nh * 512:
                                                             (nh + 1) * 512],
                                             AF.Sigmoid)
                    # vector (DVE) program order: c-chain half 0, c-chain
                    # half 1, then h mults as tanh(c) halves arrive.
                    for nh in range(2):
                        nc.vector.tensor_tensor(
                            nhs(fc, nh), sif[32:64, nh * 512:(nh + 1) * 512],
                            nhs(c_ps[32:64, :], nh), Alu.mult)
                        nc.vector.tensor_tensor(nhs(ig, nh),
                                                sif[0:32,
                                                    nh * 512:(nh + 1) * 512],
                                                nhs(tg, nh), Alu.mult)
                        nc.vector.tensor_tensor(nhs(c_ps[32:64, :], nh),
                                                nhs(fc, nh), nhs(ig, nh),
                                                Alu.add)
                    for nh in range(2):
                        nc.scalar.activation(nhs(th, nh),
                                             nhs(c_ps[32:64, :], nh), AF.Tanh)
                    for nh in range(2):
                        nc.vector.tensor_tensor(nhs(h_bf, nh), nhs(so, nh),
                                                nhs(th, nh), Alu.mult)

                    # next step's x@W prefill streams while gates run
                    if not (last and s == G - 1):
                        ns = gstep + 1
                        nt = tau + (s + 1) // G
                        issue_xw(z_ps[ns % 2], srcs[nt], ns)

                    # transpose h back to [128, 32] k-tile blocks (critical:
                    # feeds next step's stationary), then the c side lazily.
                    ps_t = tp_.tile([128, 2 * SW], f32, tag="T",
                                    name=f"pst_{tau}_{s}")
                    for j in range(KT):
                        nc.tensor.matmul(
                            ps_t[:, 32 * j:32 * (j + 1)],
                            h_bf[:, 128 * j:128 * (j + 1)],
                            eye_sb[:],
                            start=True, stop=True, skip_group_check=True)
                    nc.vector.tensor_copy(hT[gstep % 2][:], ps_t[:, 0:SW])
                    # c side: cast + transpose + transport copy, off the
                    # recurrence path (scalar does the PSUM reads)
                    nc.scalar.activation(c_bf[:], c_ps[32:64, :], AF.Copy)
                    for j in range(KT):
                        nc.tensor.matmul(
                            ps_t[:, SW + 32 * j:SW + 32 * (j + 1)],
                            c_bf[:, 128 * j:128 * (j + 1)],
                            eye_sb[:],
                            start=True, stop=True, skip_group_check=True)
                    nc.scalar.copy(
                        cbf[:, s * SW:(s + 1) * SW], ps_t[:, SW:2 * SW])
                    gstep += 1

                nc.sync.dma_start(
                    c_out[(tau // R) % 3]
                    [:, (tau % R) * CW:(tau % R + 1) * CW],
                    cbf[:])
                srcs.pop(tau, None)

            # final state out
            cfin = sp.tile([32, H], f32)
            nc.vector.tensor_copy(cfin[:], c_ps[32:64, :])
            nc.sync.dma_start(out_ext[:], cfin[:])
    nc.finalize()
    return nc


def _prep_in_maps(inputs, W, U, b):
    # layer-0 xT chunks: [128, chunk, step, k, b] per batch half
    x6 = (inputs.astype(np.float32)
          .reshape(NCH, G, B, KT, 128)
          .transpose(4, 0, 1, 3, 2))       # [128, NCH, G, KT, B]
    halves = [
        np.ascontiguousarray(x6[:, :, :, :, h * B_LOC:(h + 1) * B_LOC]
                             .reshape(128, NCH * CW))
        .astype(ml_dtypes.bfloat16)
        for h in range(2)
    ]
    zeros_src = np.zeros((128, NCH * CW), dtype=ml_dtypes.bfloat16)
    # weights: k-tile k at cols k*4H (rows 128k..128k+128 of the [H,4H] mat)
    Wk = np.ascontiguousarray(
        W.reshape(L, KT, 128, 4 * H).transpose(0, 2, 1, 3)
        .reshape(L, 128, KT * 4 * H)).astype(ml_dtypes.bfloat16)
    Uk = np.ascontiguousarray(
        U.reshape(L, KT, 128, 4 * H).transpose(0, 2, 1, 3)
        .reshape(L, 128, KT * 4 * H)).astype(ml_dtypes.bfloat16)
    eye = np.eye(32, dtype=ml_dtypes.bfloat16)
    in_maps = []
    for r in range(N_CORES):
        j = r % 4
        in_maps.append({
            "w_loc": np.ascontiguousarray(Wk[j]),
            "u_loc": np.ascontiguousarray(Uk[j]),
            "src_static": halves[r // 4] if j == 0 else zeros_src,
            "eye": eye,
            "rparam": np.array([[j]], dtype=np.int32),
        })
    return in_maps


def kernel(inputs, W, U, b):
    assert not np.any(b), "nonzero bias not implemented"
    from concourse.bass_utils import run_bass_kernel_spmd

    if "nc" not in _CACHE:
        _CACHE["nc"] = _build()
    nc = _CACHE["nc"]
    in_maps = _prep_in_maps(inputs, W, U, b)
    res = run_bass_kernel_spmd(nc, in_maps, core_ids=list(range(N_CORES)))
    c = np.zeros((B, H), dtype=np.float32)
    for half, rank in ((0, 3), (1, 7)):
        c[half * B_LOC:(half + 1) * B_LOC, :] = res.results[rank]["out"]
    return c


# revision 9
# speedup vs baseline: 1.1642x; 1.1642x over previous
"""Trainium2 Bass kernel for the 4-layer LSTM (T=128, B=64, H=1024).

Strategy: 4-stage layer pipeline x 2-way batch data-parallel = 8 cores.
Rank r: stage j = r % 4 (layer j), batch half = r // 4 (B_LOC = 32).
The two batch halves run identical, fully independent pipelines.

Transport: AllGathers serialize on the collective firmware chain at
~35-55us per op regardless of payload, so chunks are shipped in 2-tick
rounds (one gather per 2 ticks, carrying 2 chunks = 4 steps). Gather
outputs rotate through 3 buffers (8-rank shared-output, 4-rank, 8-rank)
because a collective's rewrite of its output buffer is NOT ordered
against reader DMAs — the rotation gives consumers a 2-round safe read
window. Stage j+1 consumes stage j's chunk OFF=6 ticks after
production, so a gather has ~2 ticks of slack before its first
consumer (measured optimum: OFF=6 beats 7 and 8 — extra fill ticks
cost more than the residual gather-wait stalls they remove).

Compute per tick (G=2 steps), all in transposed space (zT = [4H, B_LOC],
no per-step transposes):
  - G sequential LSTM steps: h@U accumulates on top of the pre-computed
    x@W chunk in PSUM (start=False), then the sigmoid/tanh gate chain
    runs on Scalar/Vector.
  - The NEXT tick's batched x@W matmuls are interleaved between the
    per-step h@U blocks, so the PE (in-order) streams independent work
    during the gate chains. PSUM is double-buffered (4 banks per tick).
  - Each step's cT (bf16) goes to the round's DRAM bounce slot.

Output = cell state of layer 3 at t=T-1 (rank 3 holds batch 0:32,
rank 7 holds batch 32:64).
"""

import sys

for p in ("/opt/trn_rl_repo",):
    if p not in sys.path:
        sys.path.insert(0, p)

import numpy as np
import ml_dtypes

T, B, H, L = 128, 64, 1024, 4
FH = 4 * H
KT = H // 128           # 8 K-tiles
MT = FH // 128          # 32 M-tiles
B_LOC = B // 2          # batch per core (2-way data parallel)
G = 2                   # steps per chunk (one PSUM tick)
NCH = T // G            # chunks per layer
OFF = 7                 # tick offset between consecutive stages
RR = 3                  # ticks per gather round
C0 = 2                  # first active tick of stage 0
NTICKS = NCH + OFF * (L - 1) + C0   # stage j active [C0+OFF*j, C0+OFF*j+NCH-1]
N_CORES = 8
GB = G * B_LOC          # chunk free-dim (steps x local batch)
MMPB = 512 // GB        # mm blocks per 2KB PSUM bank

_CACHE = {}


def _build(nticks=NTICKS):
    import concourse.bacc as bacc
    import concourse.bass as bass
    import concourse.mybir as mybir
    import concourse.tile as tile

    bf16, f32, i32 = mybir.dt.bfloat16, mybir.dt.float32, mybir.dt.int32
    AF = mybir.ActivationFunctionType
    Alu = mybir.AluOpType

    nc = bacc.Bacc("TRN2", target_bir_lowering=False, debug=False,
                   num_devices=N_CORES)

    w_in = nc.dram_tensor("w_loc", [H, FH], bf16, kind="ExternalInput")
    u_in = nc.dram_tensor("u_loc", [H, FH], bf16, kind="ExternalInput")
    # partition-major: row p holds [chunk, k, g, b] (512B contiguous/chunk)
    src_static = nc.dram_tensor("src_static", [128, NCH * KT * GB], bf16,
                                kind="ExternalInput")
    rparam = nc.dram_tensor("rparam", [1, 2], i32, kind="ExternalInput")
    out_ext = nc.dram_tensor("out", [128, KT * B_LOC], f32,
                             kind="ExternalOutput")

    # Round-sized DRAM bounce buffers (a round = 2 ticks = 2 chunks), all
    # partition-major ([128, sub-chunk, k, n] rows) so every transport DMA
    # moves contiguous 512B-per-partition runs instead of 64B scraps.
    # c_out double-buffered by round parity; the gather target alternates
    # by gather-round parity between the two comms.
    CW = KT * GB          # one chunk's bytes-per-partition (in elements)
    c_out = [nc.dram_tensor(f"c_out{i}", [128, RR * CW], bf16)
             for i in range(3)]
    # 3-deep gather rotation, all 4-rank intra-chip ops
    gbufs = [nc.dram_tensor(f"gath{i}", [4, 128, RR * CW], bf16)
             for i in range(3)]

    with tile.TileContext(nc) as tc:
        with (
            tc.tile_pool(name="wp", bufs=1) as wp,
            tc.tile_pool(name="sp", bufs=1) as sp,
            tc.tile_pool(name="srcp", bufs=3) as srcp,
            tc.tile_pool(name="ewp", bufs=2) as ewp,
            tc.tile_pool(name="zp", bufs=2, space="PSUM") as zp,
        ):
            # ---- preamble -------------------------------------------------
            w_sb = wp.tile([128, KT * FH], bf16)   # W K-tile k at k*FH
            u_sb = wp.tile([128, KT * FH], bf16)
            for k in range(KT):
                nc.sync.dma_start(w_sb[:, k * FH:(k + 1) * FH],
                                  w_in[k * 128:(k + 1) * 128, :])
                nc.sync.dma_start(u_sb[:, k * FH:(k + 1) * FH],
                                  u_in[k * 128:(k + 1) * 128, :])

            rp_sb = sp.tile([1, 2], i32)
            nc.sync.dma_start(rp_sb[:], rparam[:])
            rv = nc.values_load(rp_sb[:1, 0:1].to_broadcast((1, 1)))
            rk = nc.values_load(rp_sb[:1, 1:2].to_broadcast((1, 1)))

            zsb = sp.tile([128, RR * CW], bf16)
            nc.gpsimd.memset(zsb[:], 0.0)
            for cb in c_out:
                nc.sync.dma_start(cb[:, :], zsb[:])
            for gb in gbufs:
                for s in range(4):
                    nc.sync.dma_start(gb[s][:, :], zsb[:])

            # state (double-buffered by global step parity)
            cT = [sp.tile([128, KT * B_LOC], f32, name=f"cT{i}")
                  for i in range(2)]
            hT = [sp.tile([128, KT * B_LOC], bf16, name=f"hT{i}")
                  for i in range(2)]
            for i in range(2):
                nc.gpsimd.memset(cT[i][:], 0.0)
                nc.gpsimd.memset(hT[i][:], 0.0)

            # src chunk for consuming tick `tc_` (issued 2 ticks early):
            # stage 0 reads src_static chunk tc_-C0; stage j>0 reads the
            # chunk its predecessor produced at tick tc_-OFF from the
            # gather of round tp//2+1 (comm8 on even gather rounds).
            def issue_src(tc_):
                src_sb = srcp.tile([128, KT * GB], bf16, tag="src",
                                   name=f"src_{tc_}")
                kchunk = min(max(tc_ - C0, 0), NCH - 1)
                tp = max(tc_ - OFF, 0)
                sub = tp % RR
                sel = (tp // RR) % 3
                gt = gbufs[sel]
                with tc.If(rv == 0) as cmp:
                    nc.sync.dma_start(
                        src_sb[:],
                        src_static[:, kchunk * CW:(kchunk + 1) * CW])
                with cmp.Else():
                    for r in (1, 2, 3):
                        with tc.If(rv == r):
                            nc.sync.dma_start(
                                src_sb[:],
                                gt[r - 1][:, sub * CW:(sub + 1) * CW])
                return src_sb

            # batched x@W for mm tiles [mmlo, mmhi) of a chunk. PSUM
            # start/stop are bank-granular: only the first matmul touching
            # a bank carries start=True (clears the bank's has_written).
            def issue_xw(psz_t, src_t, mmlo, mmhi):
                for mm in range(mmlo, mmhi):
                    for k in range(KT):
                        nc.tensor.matmul(
                            psz_t[:, mm * GB:(mm + 1) * GB],
                            w_sb[:, k * FH + mm * 128:k * FH + (mm + 1) * 128],
                            src_t[:, k * GB:(k + 1) * GB],
                            start=(mm % MMPB == 0 and k == 0), stop=False,
                            skip_group_check=True,
                        )

            gstep = 0  # global step counter for state parity

            srcs = {0: issue_src(0), 1: issue_src(1)}
            psz_cur = zp.tile([128, MT * GB], f32, tag="Z", name="psz_0")
            issue_xw(psz_cur, srcs[0], 0, MT)

            # ---- tick loop ------------------------------------------------
            for tau in range(nticks):
                if tau % RR == 0 and tau > 0:
                    m = tau // RR
                    nc.gpsimd.collective_compute(
                        "AllGather", Alu.bypass,
                        replica_groups=[[0, 1, 2, 3], [4, 5, 6, 7]],
                        ins=[c_out[(m - 1) % 3].ap().opt()],
                        outs=[gbufs[(m - 1) % 3].ap().opt()],
                    )

                # state reset at each stage's first active tick
                if tau >= C0 and (tau - C0) % OFF == 0 and (tau - C0) // OFF < L:
                    j = (tau - C0) // OFF
                    with tc.If(rv == j):
                        nc.gpsimd.memset(cT[gstep % 2][:], 0.0)
                        nc.gpsimd.memset(hT[gstep % 2][:], 0.0)

                if tau + 2 < nticks:
                    srcs[tau + 2] = issue_src(tau + 2)
                last = tau == nticks - 1
                if not last:
                    psz_next = zp.tile([128, MT * GB], f32, tag="Z",
                                       name=f"psz_{tau + 1}")

                cbf = ewp.tile([128, CW], bf16, tag="cbf",
                               name=f"cbf_{tau}")
                for s in range(G):
                    h_prev = hT[gstep % 2]
                    c_prev = cT[gstep % 2]
                    h_new = hT[(gstep + 1) % 2]
                    c_new = cT[(gstep + 1) % 2]
                    # h @ U accumulated on top of x@W (+start=False)
                    for mm in range(MT):
                        for k in range(KT):
                            nc.tensor.matmul(
                                psz_cur[:, mm * GB + s * B_LOC:
                                        mm * GB + (s + 1) * B_LOC],
                                u_sb[:, k * FH + mm * 128:
                                     k * FH + (mm + 1) * 128],
                                h_prev[:, k * B_LOC:(k + 1) * B_LOC],
                                start=False,
                                stop=(s == G - 1 and mm % MMPB == MMPB - 1
                                      and k == KT - 1),
                                skip_group_check=True,
                            )
                    # gates: mm 0-7 = i, 8-15 = f, 16-23 = g, 24-31 = o
                    # step-s columns: strided views [mm, s*B_LOC:(s+1)*B_LOC]
                    def zview(g0, g1, s=s):
                        return psz_cur[:].rearrange(
                            "p (mm n) -> p mm n", n=GB
                        )[:, g0 * 8:g1 * 8, s * B_LOC:(s + 1) * B_LOC]
                    sif = ewp.tile([128, 2 * KT * B_LOC], f32, tag="sif",
                                   name=f"sif_{tau}_{s}")
                    tg = ewp.tile([128, KT * B_LOC], f32, tag="tg",
                                  name=f"tg_{tau}_{s}")
                    so = ewp.tile([128, KT * B_LOC], f32, tag="so",
                                  name=f"so_{tau}_{s}")
                    nc.scalar.activation(
                        sif[:].rearrange("p (mm n) -> p mm n", n=B_LOC),
                        zview(0, 2), AF.Sigmoid)
                    nc.scalar.activation(
                        tg[:].rearrange("p (mm n) -> p mm n", n=B_LOC),
                        zview(2, 3), AF.Tanh)
                    nc.scalar.activation(
                        so[:].rearrange("p (mm n) -> p mm n", n=B_LOC),
                        zview(3, 4), AF.Sigmoid)
                    fc = ewp.tile([128, KT * B_LOC], f32, tag="fc",
                                  name=f"fc_{tau}_{s}")
                    ig = ewp.tile([128, KT * B_LOC], f32, tag="ig",
                                  name=f"ig_{tau}_{s}")
                    nc.vector.tensor_tensor(fc[:], sif[:, KT * B_LOC:],
                                            c_prev[:], Alu.mult)
                    nc.vector.tensor_tensor(ig[:], sif[:, 0:KT * B_LOC],
                                            tg[:], Alu.mult)
                    nc.vector.tensor_tensor(c_new[:], fc[:], ig[:], Alu.add)
                    th = ewp.tile([128, KT * B_LOC], f32, tag="th",
                                  name=f"th_{tau}_{s}")
                    nc.scalar.activation(th[:], c_new[:], AF.Tanh)
                    nc.vector.tensor_tensor(h_new[:], so[:], th[:], Alu.mult)
                    # next tick's x@W half: queued on the PE behind this
                    # step's h@U block, it streams while the gate chain
                    # above runs on Scalar/Vector.
                    if not last:
                        half = MT // G
                        issue_xw(psz_next, srcs[tau + 1],
                                 s * half, (s + 1) * half)
                    # cast c into the tick's transport tile (strided by k)
                    nc.vector.tensor_copy(
                        cbf[:].rearrange("p (k n) -> p k n", n=GB)
                        [:, :, s * B_LOC:(s + 1) * B_LOC],
                        c_new[:].rearrange("p (k n) -> p k n", n=B_LOC))
                    gstep += 1

                # one contiguous transport DMA per tick (both steps)
                nc.sync.dma_start(
                    c_out[(tau // RR) % 3]
                    [:, (tau % RR) * CW:(tau % RR + 1) * CW],
                    cbf[:])

                if not last:
                    psz_cur = psz_next
                srcs.pop(tau, None)

            # final state out (ranks 3 and 7 hold the answer)
            nc.sync.dma_start(out_ext[:], cT[gstep % 2][:])
    nc.finalize()
    return nc


def _prep_in_maps(inputs, W, U, b):
    # partition-major src: [128, chunk, k, g, b] per batch half
    x5 = (inputs.astype(np.float32)
          .transpose(2, 0, 1)               # [H, T, B]
          .reshape(KT, 128, NCH, G, B)
          .transpose(1, 2, 0, 3, 4))        # [128, NCH, KT, G, B]
    halves = [
        np.ascontiguousarray(x5[:, :, :, :, :B_LOC]
                             .reshape(128, NCH * KT * GB))
        .astype(ml_dtypes.bfloat16),
        np.ascontiguousarray(x5[:, :, :, :, B_LOC:]
                             .reshape(128, NCH * KT * GB))
        .astype(ml_dtypes.bfloat16),
    ]
    zeros_src = np.zeros((128, NCH * KT * GB), dtype=ml_dtypes.bfloat16)
    Wb = W.astype(ml_dtypes.bfloat16)
    Ub = U.astype(ml_dtypes.bfloat16)
    in_maps = []
    for r in range(N_CORES):
        j = r % 4
        in_maps.append({
            "w_loc": np.ascontiguousarray(Wb[j]),
            "u_loc": np.ascontiguousarray(Ub[j]),
            "src_static": halves[r // 4] if j == 0 else zeros_src,
            "rparam": np.array([[j, r]], dtype=np.int32),
        })
    return in_maps


def kernel(inputs, W, U, b):
    assert not np.any(b), "nonzero bias not implemented"
    from concourse.bass_utils import run_bass_kernel_spmd

    if "nc" not in _CACHE:
        _CACHE["nc"] = _build()
    nc = _CACHE["nc"]
    in_maps = _prep_in_maps(inputs, W, U, b)
    res = run_bass_kernel_spmd(nc, in_maps, core_ids=list(range(N_CORES)))
    c = np.zeros((B, H), dtype=np.float32)
    for half, rank in ((0, 3), (1, 7)):
        ct = res.results[rank]["out"]  # [128, KT*B_LOC], k-tile k at k*B_LOC
        for k in range(KT):
            c[half * B_LOC:(half + 1) * B_LOC, k * 128:(k + 1) * 128] = \
                ct[:, k * B_LOC:(k + 1) * B_LOC].T
    return c



# revision 10
# speedup vs baseline: 1.2224x; 1.0501x over previous
"""Trainium2 Bass kernel for the 4-layer LSTM (T=128, B=64, H=1024).

Strategy: 4-stage layer pipeline x 2-way batch data-parallel = 8 cores.
Rank r: stage j = r % 4 (layer j), batch half = r // 4 (B_LOC = 32).
The two batch halves run identical, fully independent pipelines.

Transport: AllGathers serialize on the collective firmware chain at
~35-55us per op regardless of payload, so chunks are shipped in 2-tick
rounds (one gather per 2 ticks, carrying 2 chunks = 4 steps). Gather
outputs rotate through 3 buffers (8-rank shared-output, 4-rank, 8-rank)
because a collective's rewrite of its output buffer is NOT ordered
against reader DMAs — the rotation gives consumers a 2-round safe read
window. Stage j+1 consumes stage j's chunk OFF=6 ticks after
production, so a gather has ~2 ticks of slack before its first
consumer (measured optimum: OFF=6 beats 7 and 8 — extra fill ticks
cost more than the residual gather-wait stalls they remove).

Compute per tick (G=2 steps), all in transposed space (zT = [4H, B_LOC],
no per-step transposes):
  - G sequential LSTM steps: h@U accumulates on top of the pre-computed
    x@W chunk in PSUM (start=False), then the sigmoid/tanh gate chain
    runs on Scalar/Vector.
  - The NEXT tick's batched x@W matmuls are interleaved between the
    per-step h@U blocks, so the PE (in-order) streams independent work
    during the gate chains. PSUM is double-buffered (4 banks per tick).
  - Each step's cT (bf16) goes to the round's DRAM bounce slot.

Output = cell state of layer 3 at t=T-1 (rank 3 holds batch 0:32,
rank 7 holds batch 32:64).
"""

import sys

for p in ("/opt/trn_rl_repo",):
    if p not in sys.path:
        sys.path.insert(0, p)

import numpy as np
import ml_dtypes

T, B, H, L = 128, 64, 1024, 4
FH = 4 * H
KT = H // 128           # 8 K-tiles
MT = FH // 128          # 32 M-tiles
B_LOC = B // 2          # batch per core (2-way data parallel)
G = 2                   # steps per chunk (one PSUM tick)
NCH = T // G            # chunks per layer
OFF = 6                 # tick offset between consecutive stages
C0 = 2                  # first active tick of stage 0
NTICKS = NCH + OFF * (L - 1) + C0   # stage j active [C0+OFF*j, C0+OFF*j+NCH-1]
N_CORES = 8
GB = G * B_LOC          # chunk free-dim (steps x local batch)
MMPB = 512 // GB        # mm blocks per 2KB PSUM bank

_CACHE = {}


def _build(nticks=NTICKS):
    import concourse.bacc as bacc
    import concourse.bass as bass
    import concourse.mybir as mybir
    import concourse.tile as tile

    bf16, f32, i32 = mybir.dt.bfloat16, mybir.dt.float32, mybir.dt.int32
    AF = mybir.ActivationFunctionType
    Alu = mybir.AluOpType

    nc = bacc.Bacc("TRN2", target_bir_lowering=False, debug=False,
                   num_devices=N_CORES)

    w_in = nc.dram_tensor("w_loc", [H, FH], bf16, kind="ExternalInput")
    u_in = nc.dram_tensor("u_loc", [H, FH], bf16, kind="ExternalInput")
    # partition-major: row p holds [chunk, k, g, b] (512B contiguous/chunk)
    src_static = nc.dram_tensor("src_static", [128, NCH * KT * GB], bf16,
                                kind="ExternalInput")
    rparam = nc.dram_tensor("rparam", [1, 2], i32, kind="ExternalInput")
    out_ext = nc.dram_tensor("out", [128, KT * B_LOC], f32,
                             kind="ExternalOutput")

    # Round-sized DRAM bounce buffers (a round = 2 ticks = 2 chunks), all
    # partition-major ([128, sub-chunk, k, n] rows) so every transport DMA
    # moves contiguous 512B-per-partition runs instead of 64B scraps.
    # c_out double-buffered by round parity; the gather target alternates
    # by gather-round parity between the two comms.
    CW = KT * GB          # one chunk's bytes-per-partition (in elements)
    c_out = [nc.dram_tensor(f"c_out{i}", [128, 2 * CW], bf16)
             for i in range(2)]
    # 3-deep gather rotation: a buffer is rewritten 3 rounds after it was
    # written, giving OFF=8 consumers a safe 2-round read window (Tile does
    # not order a collective's rewrite against reader DMAs).
    gbufs = [
        nc.dram_tensor("gath8a", [8, 128, 2 * CW], bf16,
                       addr_space="Shared"),
        nc.dram_tensor("gath4", [4, 128, 2 * CW], bf16),
        nc.dram_tensor("gath8b", [8, 128, 2 * CW], bf16,
                       addr_space="Shared"),
    ]

    with tile.TileContext(nc) as tc:
        with (
            tc.tile_pool(name="wp", bufs=1) as wp,
            tc.tile_pool(name="sp", bufs=1) as sp,
            tc.tile_pool(name="srcp", bufs=3) as srcp,
            tc.tile_pool(name="ewp", bufs=2) as ewp,
            tc.tile_pool(name="zp", bufs=2, space="PSUM") as zp,
        ):
            # ---- preamble -------------------------------------------------
            w_sb = wp.tile([128, KT * FH], bf16)   # W K-tile k at k*FH
            u_sb = wp.tile([128, KT * FH], bf16)
            for k in range(KT):
                nc.sync.dma_start(w_sb[:, k * FH:(k + 1) * FH],
                                  w_in[k * 128:(k + 1) * 128, :])
                nc.sync.dma_start(u_sb[:, k * FH:(k + 1) * FH],
                                  u_in[k * 128:(k + 1) * 128, :])

            rp_sb = sp.tile([1, 2], i32)
            nc.sync.dma_start(rp_sb[:], rparam[:])
            rv = nc.values_load(rp_sb[:1, 0:1].to_broadcast((1, 1)))
            rk = nc.values_load(rp_sb[:1, 1:2].to_broadcast((1, 1)))

            zsb = sp.tile([128, 2 * CW], bf16)
            nc.gpsimd.memset(zsb[:], 0.0)
            for i in range(2):
                nc.sync.dma_start(c_out[i][:, :], zsb[:])
            for gb, nslots in ((gbufs[0], 8), (gbufs[1], 4), (gbufs[2], 8)):
                for s in range(nslots):
                    nc.sync.dma_start(gb[s][:, :], zsb[:])

            # state (double-buffered by global step parity)
            cT = [sp.tile([128, KT * B_LOC], f32, name=f"cT{i}")
                  for i in range(2)]
            hT = [sp.tile([128, KT * B_LOC], bf16, name=f"hT{i}")
                  for i in range(2)]
            for i in range(2):
                nc.gpsimd.memset(cT[i][:], 0.0)
                nc.gpsimd.memset(hT[i][:], 0.0)

            # src chunk for consuming tick `tc_` (issued 2 ticks early):
            # stage 0 reads src_static chunk tc_-C0; stage j>0 reads the
            # chunk its predecessor produced at tick tc_-OFF from the
            # gather of round tp//2+1 (comm8 on even gather rounds).
            def issue_src(tc_):
                src_sb = srcp.tile([128, KT * GB], bf16, tag="src",
                                   name=f"src_{tc_}")
                kchunk = min(max(tc_ - C0, 0), NCH - 1)
                tp = max(tc_ - OFF, 0)
                sub = tp % 2
                m_g = tp // 2 + 1
                sel = m_g % 3
                use8 = sel != 1
                gt = gbufs[sel]
                with tc.If(rv == 0) as cmp:
                    nc.sync.dma_start(
                        src_sb[:],
                        src_static[:, kchunk * CW:(kchunk + 1) * CW])
                with cmp.Else():
                    ranks = (1, 2, 3, 5, 6, 7) if use8 else (1, 2, 3)
                    reg = rk if use8 else rv
                    for r in ranks:
                        with tc.If(reg == r):
                            nc.sync.dma_start(
                                src_sb[:],
                                gt[r - 1][:, sub * CW:(sub + 1) * CW])
                return src_sb

            # batched x@W for mm tiles [mmlo, mmhi) of a chunk. PSUM
            # start/stop are bank-granular: only the first matmul touching
            # a bank carries start=True (clears the bank's has_written).
            def issue_xw(psz_t, src_t, mmlo, mmhi):
                for mm in range(mmlo, mmhi):
                    for k in range(KT):
                        nc.tensor.matmul(
                            psz_t[:, mm * GB:(mm + 1) * GB],
                            w_sb[:, k * FH + mm * 128:k * FH + (mm + 1) * 128],
                            src_t[:, k * GB:(k + 1) * GB],
                            start=(mm % MMPB == 0 and k == 0), stop=False,
                            skip_group_check=True,
                        )

            gstep = 0  # global step counter for state parity

            srcs = {0: issue_src(0), 1: issue_src(1)}
            psz_cur = zp.tile([128, MT * GB], f32, tag="Z", name="psz_0")
            issue_xw(psz_cur, srcs[0], 0, MT)

            # ---- tick loop ------------------------------------------------
            for tau in range(nticks):
                if tau % 2 == 0:
                    m = tau // 2
                    sel = m % 3
                    comm8 = sel != 1
                    nc.gpsimd.collective_compute(
                        "AllGather", Alu.bypass,
                        replica_groups=([[0, 1, 2, 3, 4, 5, 6, 7]] if comm8
                                        else [[0, 1, 2, 3], [4, 5, 6, 7]]),
                        ins=[c_out[(m - 1) % 2].ap().opt()],
                        outs=[gbufs[sel].ap().opt()],
                    )

                # state reset at each stage's first active tick
                if tau >= C0 and (tau - C0) % OFF == 0 and (tau - C0) // OFF < L:
                    j = (tau - C0) // OFF
                    with tc.If(rv == j):
                        nc.gpsimd.memset(cT[gstep % 2][:], 0.0)
                        nc.gpsimd.memset(hT[gstep % 2][:], 0.0)

                if tau + 2 < nticks:
                    srcs[tau + 2] = issue_src(tau + 2)
                last = tau == nticks - 1
                if not last:
                    psz_next = zp.tile([128, MT * GB], f32, tag="Z",
                                       name=f"psz_{tau + 1}")

                cbf = ewp.tile([128, CW], bf16, tag="cbf",
                               name=f"cbf_{tau}")
                for s in range(G):
                    h_prev = hT[gstep % 2]
                    c_prev = cT[gstep % 2]
                    h_new = hT[(gstep + 1) % 2]
                    c_new = cT[(gstep + 1) % 2]
                    # h @ U accumulated on top of x@W (+start=False)
                    for mm in range(MT):
                        for k in range(KT):
                            nc.tensor.matmul(
                                psz_cur[:, mm * GB + s * B_LOC:
                                        mm * GB + (s + 1) * B_LOC],
                                u_sb[:, k * FH + mm * 128:
                                     k * FH + (mm + 1) * 128],
                                h_prev[:, k * B_LOC:(k + 1) * B_LOC],
                                start=False,
                                stop=(s == G - 1 and mm % MMPB == MMPB - 1
                                      and k == KT - 1),
                                skip_group_check=True,
                            )
                    # gates: mm 0-7 = i, 8-15 = f, 16-23 = g, 24-31 = o
                    # step-s columns: strided views [mm, s*B_LOC:(s+1)*B_LOC]
                    def zview(g0, g1, s=s):
                        return psz_cur[:].rearrange(
                            "p (mm n) -> p mm n", n=GB
                        )[:, g0 * 8:g1 * 8, s * B_LOC:(s + 1) * B_LOC]
                    sif = ewp.tile([128, 2 * KT * B_LOC], f32, tag="sif",
                                   name=f"sif_{tau}_{s}")
                    tg = ewp.tile([128, KT * B_LOC], f32, tag="tg",
                                  name=f"tg_{tau}_{s}")
                    so = ewp.tile([128, KT * B_LOC], f32, tag="so",
                                  name=f"so_{tau}_{s}")
                    nc.scalar.activation(
                        sif[:].rearrange("p (mm n) -> p mm n", n=B_LOC),
                        zview(0, 2), AF.Sigmoid)
                    nc.scalar.activation(
                        tg[:].rearrange("p (mm n) -> p mm n", n=B_LOC),
                        zview(2, 3), AF.Tanh)
                    nc.scalar.activation(
                        so[:].rearrange("p (mm n) -> p mm n", n=B_LOC),
                        zview(3, 4), AF.Sigmoid)
                    fc = ewp.tile([128, KT * B_LOC], f32, tag="fc",
                                  name=f"fc_{tau}_{s}")
                    ig = ewp.tile([128, KT * B_LOC], f32, tag="ig",
                                  name=f"ig_{tau}_{s}")
                    nc.vector.tensor_tensor(fc[:], sif[:, KT * B_LOC:],
                                            c_prev[:], Alu.mult)
                    nc.vector.tensor_tensor(ig[:], sif[:, 0:KT * B_LOC],
                                            tg[:], Alu.mult)
                    nc.vector.tensor_tensor(c_new[:], fc[:], ig[:], Alu.add)
                    th = ewp.tile([128, KT * B_LOC], f32, tag="th",
                                  name=f"th_{tau}_{s}")
                    nc.scalar.activation(th[:], c_new[:], AF.Tanh)
                    nc.vector.tensor_tensor(h_new[:], so[:], th[:], Alu.mult)
                    # next tick's x@W half: queued on the PE behind this
                    # step's h@U block, it streams while the gate chain
                    # above runs on Scalar/Vector.
                    if not last:
                        half = MT // G
                        issue_xw(psz_next, srcs[tau + 1],
                                 s * half, (s + 1) * half)
                    # cast c into the tick's transport tile (strided by k)
                    nc.vector.tensor_copy(
                        cbf[:].rearrange("p (k n) -> p k n", n=GB)
                        [:, :, s * B_LOC:(s + 1) * B_LOC],
                        c_new[:].rearrange("p (k n) -> p k n", n=B_LOC))
                    gstep += 1

                # one contiguous transport DMA per tick (both steps)
                nc.sync.dma_start(
                    c_out[(tau // 2) % 2]
                    [:, (tau % 2) * CW:(tau % 2 + 1) * CW],
                    cbf[:])

                if not last:
                    psz_cur = psz_next
                srcs.pop(tau, None)

            # final state out (ranks 3 and 7 hold the answer)
            nc.sync.dma_start(out_ext[:], cT[gstep % 2][:])
    nc.finalize()
    return nc


def _prep_in_maps(inputs, W, U, b):
    # partition-major src: [128, chunk, k, g, b] per batch half
    x5 = (inputs.astype(np.float32)
          .transpose(2, 0, 1)               # [H, T, B]
          .reshape(KT, 128, NCH, G, B)
          .transpose(1, 2, 0, 3, 4))        # [128, NCH, KT, G, B]
    halves = [
        np.ascontiguousarray(x5[:, :, :, :, :B_LOC]
                             .reshape(128, NCH * KT * GB))
        .astype(ml_dtypes.bfloat16),
        np.ascontiguousarray(x5[:, :, :, :, B_LOC:]
                             .reshape(128, NCH * KT * GB))
        .astype(ml_dtypes.bfloat16),
    ]
    zeros_src = np.zeros((128, NCH * KT * GB), dtype=ml_dtypes.bfloat16)
    Wb = W.astype(ml_dtypes.bfloat16)
    Ub = U.astype(ml_dtypes.bfloat16)
    in_maps = []
    for r in range(N_CORES):
        j = r % 4
        in_maps.append({
            "w_loc": np.ascontiguousarray(Wb[j]),
            "u_loc": np.ascontiguousarray(Ub[j]),
            "src_static": halves[r // 4] if j == 0 else zeros_src,
            "rparam": np.array([[j, r]], dtype=np.int32),
        })
    return in_maps


def kernel(inputs, W, U, b):
    assert not np.any(b), "nonzero bias not implemented"
    from concourse.bass_utils import run_bass_kernel_spmd

    if "nc" not in _CACHE:
        _CACHE["nc"] = _build()
    nc = _CACHE["nc"]
    in_maps = _prep_in_maps(inputs, W, U, b)
    res = run_bass_kernel_spmd(nc, in_maps, core_ids=list(range(N_CORES)))
    c = np.zeros((B, H), dtype=np.float32)
    for half, rank in ((0, 3), (1, 7)):
        ct = res.results[rank]["out"]  # [128, KT*B_LOC], k-tile k at k*B_LOC
        for k in range(KT):
            c[half * B_LOC:(half + 1) * B_LOC, k * 128:(k + 1) * 128] = \
                ct[:, k * B_LOC:(k + 1) * B_LOC].T
    return c



# revision 11
# speedup vs baseline: 1.2439x; 1.0176x over previous
"""Trainium2 Bass kernel for the 4-layer LSTM (T=128, B=64, H=1024).

Strategy: 4-stage layer pipeline x 2-way batch data-parallel = 8 cores.
Rank r: stage j = r % 4 (layer j), batch half = r // 4 (B_LOC = 32).
The two batch halves run identical, fully independent pipelines.

Transport: AllGathers serialize on the collective firmware chain at
~35-55us per op regardless of payload, so chunks are shipped in 2-tick
rounds (one gather per 2 ticks, carrying 2 chunks = 4 steps). Gather
outputs rotate through 3 buffers (8-rank shared-output, 4-rank, 8-rank)
because a collective's rewrite of its output buffer is NOT ordered
against reader DMAs — the rotation gives consumers a 2-round safe read
window. Stage j+1 consumes stage j's chunk OFF=6 ticks after
production, so a gather has ~2 ticks of slack before its first
consumer (measured optimum: OFF=6 beats 7 and 8 — extra fill ticks
cost more than the residual gather-wait stalls they remove).

Compute per tick (G=2 steps), all in transposed space (zT = [4H, B_LOC],
no per-step transposes):
  - G sequential LSTM steps: h@U accumulates on top of the pre-computed
    x@W chunk in PSUM (start=False), then the sigmoid/tanh gate chain
    runs on Scalar/Vector.
  - The NEXT tick's batched x@W matmuls are interleaved between the
    per-step h@U blocks, so the PE (in-order) streams independent work
    during the gate chains. PSUM is double-buffered (4 banks per tick).
  - Each step's cT (bf16) goes to the round's DRAM bounce slot.

Output = cell state of layer 3 at t=T-1 (rank 3 holds batch 0:32,
rank 7 holds batch 32:64).
"""

import sys

for p in ("/opt/trn_rl_repo",):
    if p not in sys.path:
        sys.path.insert(0, p)

import numpy as np
import ml_dtypes

T, B, H, L = 128, 64, 1024, 4
FH = 4 * H
KT = H // 128           # 8 K-tiles
MT = FH // 128          # 32 M-tiles
B_LOC = B // 2          # batch per core (2-way data parallel)
G = 2                   # steps per chunk (one PSUM tick)
NCH = T // G            # chunks per layer
OFF = 5                 # tick offset between consecutive stages
C0 = 2                  # first active tick of stage 0
NTICKS = NCH + OFF * (L - 1) + C0   # stage j active [C0+OFF*j, C0+OFF*j+NCH-1]
N_CORES = 8
GB = G * B_LOC          # chunk free-dim (steps x local batch)
MMPB = 512 // GB        # mm blocks per 2KB PSUM bank

_CACHE = {}


def _build(nticks=NTICKS):
    import concourse.bacc as bacc
    import concourse.bass as bass
    import concourse.mybir as mybir
    import concourse.tile as tile

    bf16, f32, i32 = mybir.dt.bfloat16, mybir.dt.float32, mybir.dt.int32
    AF = mybir.ActivationFunctionType
    Alu = mybir.AluOpType

    nc = bacc.Bacc("TRN2", target_bir_lowering=False, debug=False,
                   num_devices=N_CORES)

    w_in = nc.dram_tensor("w_loc", [H, FH], bf16, kind="ExternalInput")
    u_in = nc.dram_tensor("u_loc", [H, FH], bf16, kind="ExternalInput")
    # partition-major: row p holds [chunk, k, g, b] (512B contiguous/chunk)
    src_static = nc.dram_tensor("src_static", [128, NCH * KT * GB], bf16,
                                kind="ExternalInput")
    rparam = nc.dram_tensor("rparam", [1, 2], i32, kind="ExternalInput")
    out_ext = nc.dram_tensor("out", [128, KT * B_LOC], f32,
                             kind="ExternalOutput")

    # Round-sized DRAM bounce buffers (a round = 2 ticks = 2 chunks), all
    # partition-major ([128, sub-chunk, k, n] rows) so every transport DMA
    # moves contiguous 512B-per-partition runs instead of 64B scraps.
    # c_out double-buffered by round parity; the gather target alternates
    # by gather-round parity between the two comms.
    CW = KT * GB          # one chunk's bytes-per-partition (in elements)
    c_out = [nc.dram_tensor(f"c_out{i}", [128, 2 * CW], bf16)
             for i in range(2)]
    # 3-deep gather rotation: a buffer is rewritten 3 rounds after it was
    # written, giving OFF=8 consumers a safe 2-round read window (Tile does
    # not order a collective's rewrite against reader DMAs).
    gbufs = [
        nc.dram_tensor("gath8a", [8, 128, 2 * CW], bf16,
                       addr_space="Shared"),
        nc.dram_tensor("gath4", [4, 128, 2 * CW], bf16),
        nc.dram_tensor("gath8b", [8, 128, 2 * CW], bf16,
                       addr_space="Shared"),
    ]

    with tile.TileContext(nc) as tc:
        with (
            tc.tile_pool(name="wp", bufs=1) as wp,
            tc.tile_pool(name="sp", bufs=1) as sp,
            tc.tile_pool(name="srcp", bufs=3) as srcp,
            tc.tile_pool(name="ewp", bufs=2) as ewp,
            tc.tile_pool(name="zp", bufs=2, space="PSUM") as zp,
        ):
            # ---- preamble -------------------------------------------------
            w_sb = wp.tile([128, KT * FH], bf16)   # W K-tile k at k*FH
            u_sb = wp.tile([128, KT * FH], bf16)
            for k in range(KT):
                nc.sync.dma_start(w_sb[:, k * FH:(k + 1) * FH],
                                  w_in[k * 128:(k + 1) * 128, :])
                nc.sync.dma_start(u_sb[:, k * FH:(k + 1) * FH],
                                  u_in[k * 128:(k + 1) * 128, :])

            rp_sb = sp.tile([1, 2], i32)
            nc.sync.dma_start(rp_sb[:], rparam[:])
            rv = nc.values_load(rp_sb[:1, 0:1].to_broadcast((1, 1)))
            rk = nc.values_load(rp_sb[:1, 1:2].to_broadcast((1, 1)))

            zsb = sp.tile([128, 2 * CW], bf16)
            nc.gpsimd.memset(zsb[:], 0.0)
            for i in range(2):
                nc.sync.dma_start(c_out[i][:, :], zsb[:])
            for gb, nslots in ((gbufs[0], 8), (gbufs[1], 4), (gbufs[2], 8)):
                for s in range(nslots):
                    nc.sync.dma_start(gb[s][:, :], zsb[:])

            # state (double-buffered by global step parity)
            cT = [sp.tile([128, KT * B_LOC], f32, name=f"cT{i}")
                  for i in range(2)]
            hT = [sp.tile([128, KT * B_LOC], bf16, name=f"hT{i}")
                  for i in range(2)]
            for i in range(2):
                nc.gpsimd.memset(cT[i][:], 0.0)
                nc.gpsimd.memset(hT[i][:], 0.0)

            # src chunk for consuming tick `tc_` (issued 2 ticks early):
            # stage 0 reads src_static chunk tc_-C0; stage j>0 reads the
            # chunk its predecessor produced at tick tc_-OFF from the
            # gather of round tp//2+1 (comm8 on even gather rounds).
            def issue_src(tc_):
                src_sb = srcp.tile([128, KT * GB], bf16, tag="src",
                                   name=f"src_{tc_}")
                kchunk = min(max(tc_ - C0, 0), NCH - 1)
                tp = max(tc_ - OFF, 0)
                sub = tp % 2
                m_g = tp // 2 + 1
                sel = m_g % 3
                use8 = sel != 1
                gt = gbufs[sel]
                with tc.If(rv == 0) as cmp:
                    nc.sync.dma_start(
                        src_sb[:],
                        src_static[:, kchunk * CW:(kchunk + 1) * CW])
                with cmp.Else():
                    ranks = (1, 2, 3, 5, 6, 7) if use8 else (1, 2, 3)
                    reg = rk if use8 else rv
                    for r in ranks:
                        with tc.If(reg == r):
                            nc.sync.dma_start(
                                src_sb[:],
                                gt[r - 1][:, sub * CW:(sub + 1) * CW])
                return src_sb

            # batched x@W for mm tiles [mmlo, mmhi) of a chunk. PSUM
            # start/stop are bank-granular: only the first matmul touching
            # a bank carries start=True (clears the bank's has_written).
            def issue_xw(psz_t, src_t, mmlo, mmhi):
                for mm in range(mmlo, mmhi):
                    for k in range(KT):
                        nc.tensor.matmul(
                            psz_t[:, mm * GB:(mm + 1) * GB],
                            w_sb[:, k * FH + mm * 128:k * FH + (mm + 1) * 128],
                            src_t[:, k * GB:(k + 1) * GB],
                            start=(mm % MMPB == 0 and k == 0), stop=False,
                            skip_group_check=True,
                        )

            gstep = 0  # global step counter for state parity

            srcs = {0: issue_src(0), 1: issue_src(1)}
            psz_cur = zp.tile([128, MT * GB], f32, tag="Z", name="psz_0")
            issue_xw(psz_cur, srcs[0], 0, MT)

            # ---- tick loop ------------------------------------------------
            for tau in range(nticks):
                if tau % 2 == 0:
                    m = tau // 2
                    sel = m % 3
                    comm8 = sel != 1
                    nc.gpsimd.collective_compute(
                        "AllGather", Alu.bypass,
                        replica_groups=([[0, 1, 2, 3, 4, 5, 6, 7]] if comm8
                                        else [[0, 1, 2, 3], [4, 5, 6, 7]]),
                        ins=[c_out[(m - 1) % 2].ap().opt()],
                        outs=[gbufs[sel].ap().opt()],
                    )

                # state reset at each stage's first active tick
                if tau >= C0 and (tau - C0) % OFF == 0 and (tau - C0) // OFF < L:
                    j = (tau - C0) // OFF
                    with tc.If(rv == j):
                        nc.gpsimd.memset(cT[gstep % 2][:], 0.0)
                        nc.gpsimd.memset(hT[gstep % 2][:], 0.0)

                if tau + 2 < nticks:
                    srcs[tau + 2] = issue_src(tau + 2)
                last = tau == nticks - 1
                if not last:
                    psz_next = zp.tile([128, MT * GB], f32, tag="Z",
                                       name=f"psz_{tau + 1}")

                cbf = ewp.tile([128, CW], bf16, tag="cbf",
                               name=f"cbf_{tau}")
                for s in range(G):
                    h_prev = hT[gstep % 2]
                    c_prev = cT[gstep % 2]
                    h_new = hT[(gstep + 1) % 2]
                    c_new = cT[(gstep + 1) % 2]
                    # h @ U accumulated on top of x@W (+start=False)
                    for mm in range(MT):
                        for k in range(KT):
                            nc.tensor.matmul(
                                psz_cur[:, mm * GB + s * B_LOC:
                                        mm * GB + (s + 1) * B_LOC],
                                u_sb[:, k * FH + mm * 128:
                                     k * FH + (mm + 1) * 128],
                                h_prev[:, k * B_LOC:(k + 1) * B_LOC],
                                start=False,
                                stop=(s == G - 1 and mm % MMPB == MMPB - 1
                                      and k == KT - 1),
                                skip_group_check=True,
                            )
                    # gates: mm 0-7 = i, 8-15 = f, 16-23 = g, 24-31 = o
                    # step-s columns: strided views [mm, s*B_LOC:(s+1)*B_LOC]
                    def zview(g0, g1, s=s):
                        return psz_cur[:].rearrange(
                            "p (mm n) -> p mm n", n=GB
                        )[:, g0 * 8:g1 * 8, s * B_LOC:(s + 1) * B_LOC]
                    sif = ewp.tile([128, 2 * KT * B_LOC], f32, tag="sif",
                                   name=f"sif_{tau}_{s}")
                    tg = ewp.tile([128, KT * B_LOC], f32, tag="tg",
                                  name=f"tg_{tau}_{s}")
                    so = ewp.tile([128, KT * B_LOC], f32, tag="so",
                                  name=f"so_{tau}_{s}")
                    nc.scalar.activation(
                        sif[:].rearrange("p (mm n) -> p mm n", n=B_LOC),
                        zview(0, 2), AF.Sigmoid)
                    nc.scalar.activation(
                        tg[:].rearrange("p (mm n) -> p mm n", n=B_LOC),
                        zview(2, 3), AF.Tanh)
                    nc.scalar.activation(
                        so[:].rearrange("p (mm n) -> p mm n", n=B_LOC),
                        zview(3, 4), AF.Sigmoid)
                    fc = ewp.tile([128, KT * B_LOC], f32, tag="fc",
                                  name=f"fc_{tau}_{s}")
                    ig = ewp.tile([128, KT * B_LOC], f32, tag="ig",
                                  name=f"ig_{tau}_{s}")
                    nc.vector.tensor_tensor(fc[:], sif[:, KT * B_LOC:],
                                            c_prev[:], Alu.mult)
                    nc.vector.tensor_tensor(ig[:], sif[:, 0:KT * B_LOC],
                                            tg[:], Alu.mult)
                    nc.vector.tensor_tensor(c_new[:], fc[:], ig[:], Alu.add)
                    th = ewp.tile([128, KT * B_LOC], f32, tag="th",
                                  name=f"th_{tau}_{s}")
                    nc.scalar.activation(th[:], c_new[:], AF.Tanh)
                    nc.vector.tensor_tensor(h_new[:], so[:], th[:], Alu.mult)
                    # next tick's x@W half: queued on the PE behind this
                    # step's h@U block, it streams while the gate chain
                    # above runs on Scalar/Vector.
                    if not last:
                        half = MT // G
                        issue_xw(psz_next, srcs[tau + 1],
                                 s * half, (s + 1) * half)
                    # cast c into the tick's transport tile (strided by k)
                    nc.vector.tensor_copy(
                        cbf[:].rearrange("p (k n) -> p k n", n=GB)
                        [:, :, s * B_LOC:(s + 1) * B_LOC],
                        c_new[:].rearrange("p (k n) -> p k n", n=B_LOC))
                    gstep += 1

                # one contiguous transport DMA per tick (both steps)
                nc.sync.dma_start(
                    c_out[(tau // 2) % 2]
                    [:, (tau % 2) * CW:(tau % 2 + 1) * CW],
                    cbf[:])

                if not last:
                    psz_cur = psz_next
                srcs.pop(tau, None)

            # final state out (ranks 3 and 7 hold the answer)
            nc.sync.dma_start(out_ext[:], cT[gstep % 2][:])
    nc.finalize()
    return nc


def _prep_in_maps(inputs, W, U, b):
    # partition-major src: [128, chunk, k, g, b] per batch half
    x5 = (inputs.astype(np.float32)
          .transpose(2, 0, 1)               # [H, T, B]
          .reshape(KT, 128, NCH, G, B)
          .transpose(1, 2, 0, 3, 4))        # [128, NCH, KT, G, B]
    halves = [
        np.ascontiguousarray(x5[:, :, :, :, :B_LOC]
                             .reshape(128, NCH * KT * GB))
        .astype(ml_dtypes.bfloat16),
        np.ascontiguousarray(x5[:, :, :, :, B_LOC:]
                             .reshape(128, NCH * KT * GB))
        .astype(ml_dtypes.bfloat16),
    ]
    zeros_src = np.zeros((128, NCH * KT * GB), dtype=ml_dtypes.bfloat16)
    Wb = W.astype(ml_dtypes.bfloat16)
    Ub = U.astype(ml_dtypes.bfloat16)
    in_maps = []
    for r in range(N_CORES):
        j = r % 4
        in_maps.append({
            "w_loc": np.ascontiguousarray(Wb[j]),
            "u_loc": np.ascontiguousarray(Ub[j]),
            "src_static": halves[r // 4] if j == 0 else zeros_src,
            "rparam": np.array([[j, r]], dtype=np.int32),
        })
    return in_maps


def kernel(inputs, W, U, b):
    assert not np.any(b), "nonzero bias not implemented"
    from concourse.bass_utils import run_bass_kernel_spmd

    if "nc" not in _CACHE:
        _CACHE["nc"] = _build()
    nc = _CACHE["nc"]
    in_maps = _prep_in_maps(inputs, W, U, b)
    res = run_bass_kernel_spmd(nc, in_maps, core_ids=list(range(N_CORES)))
    c = np.zeros((B, H), dtype=np.float32)
    for half, rank in ((0, 3), (1, 7)):
        ct = res.results[rank]["out"]  # [128, KT*B_LOC], k-tile k at k*B_LOC
        for k in range(KT):
            c[half * B_LOC:(half + 1) * B_LOC, k * 128:(k + 1) * 128] = \
                ct[:, k * B_LOC:(k + 1) * B_LOC].T
    return c



# revision 12
# speedup vs baseline: 1.2872x; 1.0348x over previous
"""Trainium2 Bass kernel for the 4-layer LSTM (T=128, B=64, H=1024).

Strategy: 4-stage layer pipeline x 2-way batch data-parallel = 8 cores.
Rank r: stage j = r % 4 (layer j), batch half = r // 4 (B_LOC = 32).
The two batch halves run identical, fully independent pipelines.

Transport: AllGathers serialize on the collective firmware chain at
~35-55us per op regardless of payload, so chunks are shipped in 2-tick
rounds (one gather per 2 ticks, carrying 2 chunks = 4 steps). Gather
outputs rotate through 3 buffers (8-rank shared-output, 4-rank, 8-rank)
because a collective's rewrite of its output buffer is NOT ordered
against reader DMAs — the rotation gives consumers a 2-round safe read
window. Stage j+1 consumes stage j's chunk OFF=6 ticks after
production, so a gather has ~2 ticks of slack before its first
consumer (measured optimum: OFF=6 beats 7 and 8 — extra fill ticks
cost more than the residual gather-wait stalls they remove).

Compute per tick (G=2 steps), all in transposed space (zT = [4H, B_LOC],
no per-step transposes):
  - G sequential LSTM steps: h@U accumulates on top of the pre-computed
    x@W chunk in PSUM (start=False), then the sigmoid/tanh gate chain
    runs on Scalar/Vector.
  - The NEXT tick's batched x@W matmuls are interleaved between the
    per-step h@U blocks, so the PE (in-order) streams independent work
    during the gate chains. PSUM is double-buffered (4 banks per tick).
  - Each step's cT (bf16) goes to the round's DRAM bounce slot.

Output = cell state of layer 3 at t=T-1 (rank 3 holds batch 0:32,
rank 7 holds batch 32:64).
"""

import sys

for p in ("/opt/trn_rl_repo",):
    if p not in sys.path:
        sys.path.insert(0, p)

import numpy as np
import ml_dtypes

T, B, H, L = 128, 64, 1024, 4
FH = 4 * H
KT = H // 128           # 8 K-tiles
MT = FH // 128          # 32 M-tiles
B_LOC = B // 2          # batch per core (2-way data parallel)
G = 2                   # steps per chunk (one PSUM tick)
NCH = T // G            # chunks per layer
OFF = 4                 # tick offset between consecutive stages
C0 = 2                  # first active tick of stage 0
NTICKS = NCH + OFF * (L - 1) + C0   # stage j active [C0+OFF*j, C0+OFF*j+NCH-1]
N_CORES = 8
GB = G * B_LOC          # chunk free-dim (steps x local batch)
MMPB = 512 // GB        # mm blocks per 2KB PSUM bank

_CACHE = {}


def _build(nticks=NTICKS):
    import concourse.bacc as bacc
    import concourse.bass as bass
    import concourse.mybir as mybir
    import concourse.tile as tile

    bf16, f32, i32 = mybir.dt.bfloat16, mybir.dt.float32, mybir.dt.int32
    AF = mybir.ActivationFunctionType
    Alu = mybir.AluOpType

    nc = bacc.Bacc("TRN2", target_bir_lowering=False, debug=False,
                   num_devices=N_CORES)

    w_in = nc.dram_tensor("w_loc", [H, FH], bf16, kind="ExternalInput")
    u_in = nc.dram_tensor("u_loc", [H, FH], bf16, kind="ExternalInput")
    # partition-major: row p holds [chunk, k, g, b] (512B contiguous/chunk)
    src_static = nc.dram_tensor("src_static", [128, NCH * KT * GB], bf16,
                                kind="ExternalInput")
    rparam = nc.dram_tensor("rparam", [1, 2], i32, kind="ExternalInput")
    out_ext = nc.dram_tensor("out", [128, KT * B_LOC], f32,
                             kind="ExternalOutput")

    # Round-sized DRAM bounce buffers (a round = 2 ticks = 2 chunks), all
    # partition-major ([128, sub-chunk, k, n] rows) so every transport DMA
    # moves contiguous 512B-per-partition runs instead of 64B scraps.
    # c_out double-buffered by round parity; the gather target alternates
    # by gather-round parity between the two comms.
    CW = KT * GB          # one chunk's bytes-per-partition (in elements)
    c_out = [nc.dram_tensor(f"c_out{i}", [128, 2 * CW], bf16)
             for i in range(2)]
    # 3-deep gather rotation: a buffer is rewritten 3 rounds after it was
    # written, giving OFF=8 consumers a safe 2-round read window (Tile does
    # not order a collective's rewrite against reader DMAs).
    gbufs = [
        nc.dram_tensor("gath8a", [8, 128, 2 * CW], bf16,
                       addr_space="Shared"),
        nc.dram_tensor("gath4", [4, 128, 2 * CW], bf16),
        nc.dram_tensor("gath8b", [8, 128, 2 * CW], bf16,
                       addr_space="Shared"),
    ]

    with tile.TileContext(nc) as tc:
        with (
            tc.tile_pool(name="wp", bufs=1) as wp,
            tc.tile_pool(name="sp", bufs=1) as sp,
            tc.tile_pool(name="srcp", bufs=3) as srcp,
            tc.tile_pool(name="ewp", bufs=2) as ewp,
            tc.tile_pool(name="zp", bufs=2, space="PSUM") as zp,
        ):
            # ---- preamble -------------------------------------------------
            w_sb = wp.tile([128, KT * FH], bf16)   # W K-tile k at k*FH
            u_sb = wp.tile([128, KT * FH], bf16)
            for k in range(KT):
                nc.sync.dma_start(w_sb[:, k * FH:(k + 1) * FH],
                                  w_in[k * 128:(k + 1) * 128, :])
                nc.sync.dma_start(u_sb[:, k * FH:(k + 1) * FH],
                                  u_in[k * 128:(k + 1) * 128, :])

            rp_sb = sp.tile([1, 2], i32)
            nc.sync.dma_start(rp_sb[:], rparam[:])
            rv = nc.values_load(rp_sb[:1, 0:1].to_broadcast((1, 1)))
            rk = nc.values_load(rp_sb[:1, 1:2].to_broadcast((1, 1)))

            zsb = sp.tile([128, 2 * CW], bf16)
            nc.gpsimd.memset(zsb[:], 0.0)
            for i in range(2):
                nc.sync.dma_start(c_out[i][:, :], zsb[:])
            for gb, nslots in ((gbufs[0], 8), (gbufs[1], 4), (gbufs[2], 8)):
                for s in range(nslots):
                    nc.sync.dma_start(gb[s][:, :], zsb[:])

            # state (double-buffered by global step parity)
            cT = [sp.tile([128, KT * B_LOC], f32, name=f"cT{i}")
                  for i in range(2)]
            hT = [sp.tile([128, KT * B_LOC], bf16, name=f"hT{i}")
                  for i in range(2)]
            for i in range(2):
                nc.gpsimd.memset(cT[i][:], 0.0)
                nc.gpsimd.memset(hT[i][:], 0.0)

            # src chunk for consuming tick `tc_` (issued 2 ticks early):
            # stage 0 reads src_static chunk tc_-C0; stage j>0 reads the
            # chunk its predecessor produced at tick tc_-OFF from the
            # gather of round tp//2+1 (comm8 on even gather rounds).
            def issue_src(tc_):
                src_sb = srcp.tile([128, KT * GB], bf16, tag="src",
                                   name=f"src_{tc_}")
                kchunk = min(max(tc_ - C0, 0), NCH - 1)
                tp = max(tc_ - OFF, 0)
                sub = tp % 2
                m_g = tp // 2 + 1
                sel = m_g % 3
                use8 = sel != 1
                gt = gbufs[sel]
                with tc.If(rv == 0) as cmp:
                    nc.sync.dma_start(
                        src_sb[:],
                        src_static[:, kchunk * CW:(kchunk + 1) * CW])
                with cmp.Else():
                    ranks = (1, 2, 3, 5, 6, 7) if use8 else (1, 2, 3)
                    reg = rk if use8 else rv
                    for r in ranks:
                        with tc.If(reg == r):
                            nc.sync.dma_start(
                                src_sb[:],
                                gt[r - 1][:, sub * CW:(sub + 1) * CW])
                return src_sb

            # batched x@W for mm tiles [mmlo, mmhi) of a chunk. PSUM
            # start/stop are bank-granular: only the first matmul touching
            # a bank carries start=True (clears the bank's has_written).
            def issue_xw(psz_t, src_t, mmlo, mmhi):
                for mm in range(mmlo, mmhi):
                    for k in range(KT):
                        nc.tensor.matmul(
                            psz_t[:, mm * GB:(mm + 1) * GB],
                            w_sb[:, k * FH + mm * 128:k * FH + (mm + 1) * 128],
                            src_t[:, k * GB:(k + 1) * GB],
                            start=(mm % MMPB == 0 and k == 0), stop=False,
                            skip_group_check=True,
                        )

            gstep = 0  # global step counter for state parity

            srcs = {0: issue_src(0), 1: issue_src(1)}
            psz_cur = zp.tile([128, MT * GB], f32, tag="Z", name="psz_0")
            issue_xw(psz_cur, srcs[0], 0, MT)

            # ---- tick loop ------------------------------------------------
            for tau in range(nticks):
                if tau % 2 == 0:
                    m = tau // 2
                    sel = m % 3
                    comm8 = sel != 1
                    nc.gpsimd.collective_compute(
                        "AllGather", Alu.bypass,
                        replica_groups=([[0, 1, 2, 3, 4, 5, 6, 7]] if comm8
                                        else [[0, 1, 2, 3], [4, 5, 6, 7]]),
                        ins=[c_out[(m - 1) % 2].ap().opt()],
                        outs=[gbufs[sel].ap().opt()],
                    )

                # state reset at each stage's first active tick
                if tau >= C0 and (tau - C0) % OFF == 0 and (tau - C0) // OFF < L:
                    j = (tau - C0) // OFF
                    with tc.If(rv == j):
                        nc.gpsimd.memset(cT[gstep % 2][:], 0.0)
                        nc.gpsimd.memset(hT[gstep % 2][:], 0.0)

                if tau + 2 < nticks:
                    srcs[tau + 2] = issue_src(tau + 2)
                last = tau == nticks - 1
                if not last:
                    psz_next = zp.tile([128, MT * GB], f32, tag="Z",
                                       name=f"psz_{tau + 1}")

                cbf = ewp.tile([128, CW], bf16, tag="cbf",
                               name=f"cbf_{tau}")
                for s in range(G):
                    h_prev = hT[gstep % 2]
                    c_prev = cT[gstep % 2]
                    h_new = hT[(gstep + 1) % 2]
                    c_new = cT[(gstep + 1) % 2]
                    # h @ U accumulated on top of x@W (+start=False)
                    for mm in range(MT):
                        for k in range(KT):
                            nc.tensor.matmul(
                                psz_cur[:, mm * GB + s * B_LOC:
                                        mm * GB + (s + 1) * B_LOC],
                                u_sb[:, k * FH + mm * 128:
                                     k * FH + (mm + 1) * 128],
                                h_prev[:, k * B_LOC:(k + 1) * B_LOC],
                                start=False,
                                stop=(s == G - 1 and mm % MMPB == MMPB - 1
                                      and k == KT - 1),
                                skip_group_check=True,
                            )
                    # gates: mm 0-7 = i, 8-15 = f, 16-23 = g, 24-31 = o
                    # step-s columns: strided views [mm, s*B_LOC:(s+1)*B_LOC]
                    def zview(g0, g1, s=s):
                        return psz_cur[:].rearrange(
                            "p (mm n) -> p mm n", n=GB
                        )[:, g0 * 8:g1 * 8, s * B_LOC:(s + 1) * B_LOC]
                    sif = ewp.tile([128, 2 * KT * B_LOC], f32, tag="sif",
                                   name=f"sif_{tau}_{s}")
                    tg = ewp.tile([128, KT * B_LOC], f32, tag="tg",
                                  name=f"tg_{tau}_{s}")
                    so = ewp.tile([128, KT * B_LOC], f32, tag="so",
                                  name=f"so_{tau}_{s}")
                    nc.scalar.activation(
                        sif[:].rearrange("p (mm n) -> p mm n", n=B_LOC),
                        zview(0, 2), AF.Sigmoid)
                    nc.scalar.activation(
                        tg[:].rearrange("p (mm n) -> p mm n", n=B_LOC),
                        zview(2, 3), AF.Tanh)
                    nc.scalar.activation(
                        so[:].rearrange("p (mm n) -> p mm n", n=B_LOC),
                        zview(3, 4), AF.Sigmoid)
                    fc = ewp.tile([128, KT * B_LOC], f32, tag="fc",
                                  name=f"fc_{tau}_{s}")
                    ig = ewp.tile([128, KT * B_LOC], f32, tag="ig",
                                  name=f"ig_{tau}_{s}")
                    nc.vector.tensor_tensor(fc[:], sif[:, KT * B_LOC:],
                                            c_prev[:], Alu.mult)
                    nc.vector.tensor_tensor(ig[:], sif[:, 0:KT * B_LOC],
                                            tg[:], Alu.mult)
                    nc.vector.tensor_tensor(c_new[:], fc[:], ig[:], Alu.add)
                    th = ewp.tile([128, KT * B_LOC], f32, tag="th",
                                  name=f"th_{tau}_{s}")
                    nc.scalar.activation(th[:], c_new[:], AF.Tanh)
                    nc.vector.tensor_tensor(h_new[:], so[:], th[:], Alu.mult)
                    # next tick's x@W half: queued on the PE behind this
                    # step's h@U block, it streams while the gate chain
                    # above runs on Scalar/Vector.
                    if not last:
                        half = MT // G
                        issue_xw(psz_next, srcs[tau + 1],
                                 s * half, (s + 1) * half)
                    # cast c into the tick's transport tile (strided by k)
                    nc.vector.tensor_copy(
                        cbf[:].rearrange("p (k n) -> p k n", n=GB)
                        [:, :, s * B_LOC:(s + 1) * B_LOC],
                        c_new[:].rearrange("p (k n) -> p k n", n=B_LOC))
                    gstep += 1

                # one contiguous transport DMA per tick (both steps)
                nc.sync.dma_start(
                    c_out[(tau // 2) % 2]
                    [:, (tau % 2) * CW:(tau % 2 + 1) * CW],
                    cbf[:])

                if not last:
                    psz_cur = psz_next
                srcs.pop(tau, None)

            # final state out (ranks 3 and 7 hold the answer)
            nc.sync.dma_start(out_ext[:], cT[gstep % 2][:])
    nc.finalize()
    return nc


def _prep_in_maps(inputs, W, U, b):
    # partition-major src: [128, chunk, k, g, b] per batch half
    x5 = (inputs.astype(np.float32)
          .transpose(2, 0, 1)               # [H, T, B]
          .reshape(KT, 128, NCH, G, B)
          .transpose(1, 2, 0, 3, 4))        # [128, NCH, KT, G, B]
    halves = [
        np.ascontiguousarray(x5[:, :, :, :, :B_LOC]
                             .reshape(128, NCH * KT * GB))
        .astype(ml_dtypes.bfloat16),
        np.ascontiguousarray(x5[:, :, :, :, B_LOC:]
                             .reshape(128, NCH * KT * GB))
        .astype(ml_dtypes.bfloat16),
    ]
    zeros_src = np.zeros((128, NCH * KT * GB), dtype=ml_dtypes.bfloat16)
    Wb = W.astype(ml_dtypes.bfloat16)
    Ub = U.astype(ml_dtypes.bfloat16)
    in_maps = []
    for r in range(N_CORES):
        j = r % 4
        in_maps.append({
            "w_loc": np.ascontiguousarray(Wb[j]),
            "u_loc": np.ascontiguousarray(Ub[j]),
            "src_static": halves[r // 4] if j == 0 else zeros_src,
            "rparam": np.array([[j, r]], dtype=np.int32),
        })
    return in_maps


def kernel(inputs, W, U, b):
    assert not np.any(b), "nonzero bias not implemented"
    from concourse.bass_utils import run_bass_kernel_spmd

    if "nc" not in _CACHE:
        _CACHE["nc"] = _build()
    nc = _CACHE["nc"]
    in_maps = _prep_in_maps(inputs, W, U, b)
    res = run_bass_kernel_spmd(nc, in_maps, core_ids=list(range(N_CORES)))
    c = np.zeros((B, H), dtype=np.float32)
    for half, rank in ((0, 3), (1, 7)):
        ct = res.results[rank]["out"]  # [128, KT*B_LOC], k-tile k at k*B_LOC
        for k in range(KT):
            c[half * B_LOC:(half + 1) * B_LOC, k * 128:(k + 1) * 128] = \
                ct[:, k * B_LOC:(k + 1) * B_LOC].T
    return c

